# revision 2
# baseline (speedup 1.0000x reference)
"""NeuralCDE Bass kernel for Trainium2 (8 NeuronCores, data parallel).

Computes the reference NeuralCDE: cubic-spline-controlled ODE integrated with
torchdiffeq-style rk4 (3/8 rule) over 254 fixed steps, MLP vector field
(32 -> 128 -> 128 -> 32x8 with tanh), initial MLP and readout MLP.

Layout (per core, batch BC=2048):
  - batch split into 8 subchunks of 256; groups G0 = subchunks 0..3,
    G1 = 4..7.  Two groups are interleaved so the serial RK4 dependency
    chain of one group overlaps the other group's work on other engines.
  - activations are feature-major: z packed as (128, 256) tiles with row
    32*q + h (q = local subchunk, h = hidden dim), batch along free dim.
  - MLP: PE matmuls (row-packed for K<=32), tanh on ACT with fused bias.
  - spline derivative: XdotRep = [R; f R; f^2 R]^T @ coeff-slab on PE
    (R replicates channel c to all rows with row % 8 == c), multiplied
    into tanh(F) on DVE, then channel-summed via 0/1 matmuls on PE.
  - rk4 z-updates on DVE via scalar_tensor_tensor using identities that
    consume each k_i immediately:
       z2 = z + (dt/3) k1
       z3 = dt*k2 + (2z - z2)
       z4 = dt*k3 + (2*z2 - z3)
       z5 = ((dt*k4 + (3*z4 + (6*z3 - z)))) / 8
"""

import sys
import time

sys.path.insert(0, "/opt/trn_rl_repo")

import numpy as np

import concourse.bacc as bacc
import concourse.bass as bass
from concourse import bass_utils, mybir, tile

F32 = mybir.dt.float32
AF = mybir.ActivationFunctionType
OP = mybir.AluOpType

CORES = 8
B = 16384
BC = B // CORES          # 2048 batch per core
SUB = 512                # subchunk batch
NSUB_G = 2               # subchunks per group
GN = SUB * NSUB_G        # 1024 batch per group
L = 128                  # knots
NPIECE = L - 1           # 127
STEPS = 2 * (L - 1)      # 254
HID = 32
C = 8
DT = 0.5

_BUILD_CACHE = {}


def _schedule(num_steps):
    """Per (step, eval) -> (piece, frac_index); frac = fi/6."""
    sched = []
    for s in range(num_steps):
        evs = []
        for e in range(4):
            T = 3 * s + e  # time in units of 1/6 is T... (t = T/6? t0=s/2, offsets 0,1/6,1/3,1/2 -> T=3s+e sixths)
            idx = min(T // 6, NPIECE - 1)
            fi = T - 6 * idx
            evs.append((idx, fi))
        sched.append(evs)
    return sched


def _build(num_steps=STEPS, n_pieces=NPIECE, n_knots=L):
    key = (num_steps, n_pieces, n_knots)
    if key in _BUILD_CACHE:
        return _BUILD_CACHE[key]

    t_start = time.time()
    nc = bacc.Bacc("TRN2", target_bir_lowering=False, debug=False)

    # ---------------- DRAM I/O ----------------
    csA_d = nc.dram_tensor("csA", (n_pieces, 64, SUB), F32, kind="ExternalInput")
    csB_d = nc.dram_tensor("csB", (n_pieces, 64, SUB), F32, kind="ExternalInput")
    a0_d = nc.dram_tensor("a0", (2, 64, SUB), F32, kind="ExternalInput")
    wf1_d = nc.dram_tensor("wf1", (128, 128), F32, kind="ExternalInput")   # tile(fW1,(4,1))
    wf2_d = nc.dram_tensor("wf2", (128, 128), F32, kind="ExternalInput")
    wf3_d = nc.dram_tensor("wf3", (128, 256), F32, kind="ExternalInput")
    w0_d = nc.dram_tensor("w0", (128, 7 * 128), F32, kind="ExternalInput")  # rep mats per frac idx
    sab_d = nc.dram_tensor("sab", (128, 64), F32, kind="ExternalInput")
    wi1_d = nc.dram_tensor("wi1", (128, 64), F32, kind="ExternalInput")
    wi2_d = nc.dram_tensor("wi2", (128, 32), F32, kind="ExternalInput")
    wr1_d = nc.dram_tensor("wr1", (128, 32), F32, kind="ExternalInput")
    wr2_d = nc.dram_tensor("wr2", (128, 32), F32, kind="ExternalInput")
    fb1_d = nc.dram_tensor("fb1", (128, 1), F32, kind="ExternalInput")
    fb2_d = nc.dram_tensor("fb2", (128, 1), F32, kind="ExternalInput")
    fb3_d = nc.dram_tensor("fb3", (128, 2), F32, kind="ExternalInput")
    ib1_d = nc.dram_tensor("ib1", (64, 1), F32, kind="ExternalInput")
    ib2_d = nc.dram_tensor("ib2", (128, 1), F32, kind="ExternalInput")
    rb1_d = nc.dram_tensor("rb1", (128, 1), F32, kind="ExternalInput")
    out_d = nc.dram_tensor("out", (2, 128, 2, SUB), F32, kind="ExternalOutput")

    sched = _schedule(num_steps)

    with tile.TileContext(nc) as tc:
        with (
            tc.tile_pool(name="wpool", bufs=1) as wpool,
            tc.tile_pool(name="cs", bufs=3) as cspool,
            tc.tile_pool(name="zp", bufs=2) as zpool,
            tc.tile_pool(name="hp", bufs=2) as hpool,
            tc.tile_pool(name="fp", bufs=2) as fpool,
            tc.tile_pool(name="mlp_ps", bufs=2, space="PSUM") as mlp_ps,
            tc.tile_pool(name="f3_ps", bufs=1, space="PSUM") as f3_ps,
            tc.tile_pool(name="rep_ps", bufs=1, space="PSUM") as rep_ps,
            tc.tile_pool(name="k_ps", bufs=1, space="PSUM") as k_ps,
        ):
            _tn = [0]

            def mktile(pool, shape, tag):
                _tn[0] += 1
                return pool.tile(list(shape), F32, tag=tag, name=f"{tag}_{_tn[0]}")

            # ---------------- load weights ----------------
            def wtile(dram, shape):
                t = mktile(wpool, shape, dram.name + "_t")
                nc.sync.dma_start(t[:], dram.ap())
                return t

            wf1 = wtile(wf1_d, (128, 128))
            wf2 = wtile(wf2_d, (128, 128))
            wf3 = wtile(wf3_d, (128, 256))
            w0 = wtile(w0_d, (128, 7 * 128))
            sab = wtile(sab_d, (128, 64))
            wi1 = wtile(wi1_d, (128, 64))
            wi2 = wtile(wi2_d, (128, 32))
            wr1 = wtile(wr1_d, (128, 32))
            wr2 = wtile(wr2_d, (128, 32))
            fb1 = wtile(fb1_d, (128, 1))
            fb2 = wtile(fb2_d, (128, 1))
            fb3 = wtile(fb3_d, (128, 2))
            ib1 = wtile(ib1_d, (64, 1))
            ib2 = wtile(ib2_d, (128, 1))
            rb1 = wtile(rb1_d, (128, 1))
            a0 = [None, None]
            for g in range(2):
                a0[g] = mktile(wpool, [64, SUB], f"a0_{g}")
                nc.sync.dma_start(a0[g][:], a0_d.ap()[g])
            zeros = mktile(wpool, [64, SUB], "zeros")
            nc.gpsimd.memset(zeros[:], 0.0)

            # ---------------- coefficient slab prefetch ----------------
            cs_dram = [csA_d, csB_d]
            cs_tiles = [{}, {}]

            def load_piece(p):
                if p >= n_pieces:
                    return
                for g in range(2):
                    t = mktile(cspool, [64, SUB], f"cs{g}")
                    nc.sync.dma_start(t[:], cs_dram[g].ap()[p])
                    cs_tiles[g][p] = t

            for p in range(min(3, n_pieces)):
                load_piece(p)

            # ---------------- z0 init ----------------
            z = [None, None]
            for g in range(2):
                zi_ps = mktile(rep_ps, [64, SUB], "rep")
                h0ps = mktile(mlp_ps, [128, GN], "mlp")
                h0 = mktile(hpool, [128, GN], "h0")
                for q in range(2):
                    nc.tensor.matmul(
                        h0ps[64 * q:64 * q + 64, q * SUB:(q + 1) * SUB],
                        wi1[32 * q:32 * q + 8, 0:64],
                        a0[g][32 * q:32 * q + 8, :],
                        tile_position=(32 * q, 64 * q),
                    )
                    nc.scalar.activation(
                        h0[64 * q:64 * q + 64, q * SUB:(q + 1) * SUB],
                        h0ps[64 * q:64 * q + 64, q * SUB:(q + 1) * SUB],
                        AF.Relu, bias=ib1[:],
                    )
                    nc.tensor.matmul(
                        zi_ps[32 * q:32 * q + 32, :],
                        wi2[64 * q:64 * q + 64, 0:32],
                        h0[64 * q:64 * q + 64, q * SUB:(q + 1) * SUB],
                        tile_position=(64 * q, 32 * q),
                    )
                zg = mktile(zpool, [64, SUB], f"z{g}")
                nc.scalar.activation(zg[:], zi_ps[:], AF.Identity, bias=ib2[0:64, :])
                z[g] = zg

            # ---------------- readout ----------------
            def readout(g, ztile, l):
                r1_ps = mktile(f3_ps, [64, SUB], "f3")
                for q in range(2):
                    nc.tensor.matmul(
                        r1_ps[32 * q:32 * q + 32, :],
                        wr1[32 * q:32 * q + 32, :],
                        ztile[32 * q:32 * q + 32, :],
                        tile_position=(32 * q, 32 * q),
                    )
                r1 = mktile(hpool, [64, SUB], "r1")
                nc.vector.scalar_tensor_tensor(
                    r1[:], r1_ps[:], rb1[0:64, :], zeros[:], OP.add, OP.max
                )
                o_ps = mktile(rep_ps, [64, SUB], "rep")
                for q in range(2):
                    nc.tensor.matmul(
                        o_ps[32 * q:32 * q + 32, :],
                        wr2[32 * q:32 * q + 32, :],
                        r1[32 * q:32 * q + 32, :],
                        tile_position=(32 * q, 32 * q),
                    )
                osb = mktile(hpool, [64, SUB], "osb")
                nc.vector.tensor_copy(osb[:], o_ps[:])
                src = osb.rearrange("(q r) n -> q r n", r=32)[:, 0, :]
                nc.sync.dma_start(out_d.ap()[g, l], src)

            for g in range(2):
                readout(g, z[g], 0)

            # ---------------- one eval of g(t, z) ----------------
            def emit_eval(g, z_in, piece, fi):
                """returns kacc psum tile (64, 512), k packed like z."""
                cs = cs_tiles[g][piece]
                # mm1 + tanh  (2 row-group-packed matmuls, separate banks)
                h1ps = mktile(mlp_ps, [128, GN], "mlp")
                for q in range(2):
                    nc.tensor.matmul(
                        h1ps[:, q * SUB:(q + 1) * SUB],
                        wf1[32 * q:32 * q + 32, :],
                        z_in[32 * q:32 * q + 32, :],
                        tile_position=(32 * q, 0),
                    )
                h1 = mktile(hpool, [128, GN], f"h1_{g}")
                nc.scalar.activation(h1[:], h1ps[:], AF.Tanh, bias=fb1[:])
                # mm2 + tanh
                h2ps = mktile(mlp_ps, [128, GN], "mlp")
                for n2 in range(2):
                    nc.tensor.matmul(
                        h2ps[:, n2 * 512:(n2 + 1) * 512],
                        wf2[:, :],
                        h1[:, n2 * 512:(n2 + 1) * 512],
                    )
                h2 = mktile(hpool, [128, GN], f"h2_{g}")
                nc.scalar.activation(h2[:], h2ps[:], AF.Tanh, bias=fb2[:])
                # mm3 + tanh (two M-halves, serialized on one psum slot)
                F = []
                for t in range(2):
                    f3p = mktile(f3_ps, [128, GN], "f3")
                    for n2 in range(2):
                        nc.tensor.matmul(
                            f3p[:, n2 * 512:(n2 + 1) * 512],
                            wf3[:, t * 128:(t + 1) * 128],
                            h2[:, n2 * 512:(n2 + 1) * 512],
                        )
                    Ft = mktile(fpool, [128, GN], f"F_{g}")
                    nc.scalar.activation(
                        Ft[:], f3p[:], AF.Tanh, bias=fb3[:, t:t + 1]
                    )
                    F.append(Ft)
                # Xdot replication + multiply, one batch half at a time
                P = [mktile(fpool, [128, GN], f"P_{g}") for _ in range(2)]
                for q in range(2):
                    rep = mktile(rep_ps, [128, SUB], "rep")
                    nc.tensor.matmul(
                        rep[:, :],
                        w0[32 * q:32 * q + 24, fi * 128:(fi + 1) * 128],
                        cs[32 * q:32 * q + 24, :],
                        tile_position=(32 * q, 0),
                    )
                    for t in range(2):
                        nc.vector.tensor_tensor(
                            P[t][:, q * SUB:(q + 1) * SUB],
                            F[t][:, q * SUB:(q + 1) * SUB],
                            rep[:, :],
                            OP.mult,
                        )
                # grouped channel sum -> k
                kacc = mktile(k_ps, [64, SUB], "kacc")
                for q in range(2):
                    nc.tensor.matmul(
                        kacc[32 * q:32 * q + 32, :],
                        sab[:, 0:32],
                        P[0][:, q * SUB:(q + 1) * SUB],
                        start=True, stop=False,
                        tile_position=(0, 32 * q),
                    )
                    nc.tensor.matmul(
                        kacc[32 * q:32 * q + 32, :],
                        sab[:, 32:64],
                        P[1][:, q * SUB:(q + 1) * SUB],
                        start=False, stop=True,
                        tile_position=(0, 32 * q),
                    )
                return kacc

            # ---------------- main time loop ----------------
            STT = nc.vector.scalar_tensor_tensor
            for s in range(num_steps):
                if s % 2 == 0:
                    load_piece(s // 2 + 3)
                ksched = sched[s]
                z2 = [None, None]
                z3 = [None, None]
                z4 = [None, None]
                znew = [None, None]
                # eval 1
                for g in range(2):
                    k1 = emit_eval(g, z[g], *ksched[0])
                    z2[g] = mktile(zpool, [64, SUB], f"z2_{g}")
                    STT(z2[g][:], k1[:], DT / 3.0, z[g][:], OP.mult, OP.add)
                # eval 2
                for g in range(2):
                    k2 = emit_eval(g, z2[g], *ksched[1])
                    tmp = mktile(zpool, [64, SUB], f"tmp_{g}")
                    STT(tmp[:], z[g][:], 2.0, z2[g][:], OP.mult, OP.subtract)
                    z3[g] = mktile(zpool, [64, SUB], f"z3_{g}")
                    STT(z3[g][:], k2[:], DT, tmp[:], OP.mult, OP.add)
                # eval 3
                for g in range(2):
                    k3 = emit_eval(g, z3[g], *ksched[2])
                    tmp2 = mktile(zpool, [64, SUB], f"tmp2_{g}")
                    STT(tmp2[:], z2[g][:], 2.0, z3[g][:], OP.mult, OP.subtract)
                    z4[g] = mktile(zpool, [64, SUB], f"z4_{g}")
                    STT(z4[g][:], k3[:], DT, tmp2[:], OP.mult, OP.add)
                # eval 4
                for g in range(2):
                    k4 = emit_eval(g, z4[g], *ksched[3])
                    t3 = mktile(zpool, [64, SUB], f"t3_{g}")
                    STT(t3[:], z3[g][:], 6.0, z[g][:], OP.mult, OP.subtract)
                    t4 = mktile(zpool, [64, SUB], f"t4_{g}")
                    STT(t4[:], z4[g][:], 3.0, t3[:], OP.mult, OP.add)
                    u = mktile(zpool, [64, SUB], f"u_{g}")
                    STT(u[:], k4[:], DT, t4[:], OP.mult, OP.add)
                    znew[g] = mktile(zpool, [64, SUB], f"z{g}")
                    nc.vector.tensor_scalar_mul(znew[g][:], u[:], 0.125)
                    z[g] = znew[g]
                if s % 2 == 1:
                    l = (s + 1) // 2
                    if l < n_knots:
                        for g in range(2):
                            readout(g, z[g], l)


    t_trace = time.time()
    nc.compile()
    t_compile = time.time()
    print(f"[kernel] trace {t_trace - t_start:.1f}s, "
          f"tile-schedule+compile {t_compile - t_trace:.1f}s, "
          f"instructions: {sum(len(b.instructions) for f in nc.m.functions for b in f.blocks)}")
    _BUILD_CACHE[key] = nc
    return nc


# =====================================================================
# host-side data prep
# =====================================================================

def _prep_weights(iW1, ib1, iW2, ib2, fW1, fb1, fW2, fb2, fW3, fb3, rW1, rb1, rW2):
    R = np.zeros((C, 128), np.float32)
    for j in range(128):
        R[j % C, j] = 1.0
    w0 = np.zeros((128, 7 * 128), np.float32)
    for fi in range(7):
        f = fi / 6.0
        blk = np.concatenate([R, f * R, f * f * R, np.zeros((8, 128), np.float32)], axis=0)  # (32,128)
        w0[:, fi * 128:(fi + 1) * 128] = np.tile(blk, (4, 1))
    sab = np.zeros((128, 64), np.float32)
    for j in range(128):
        sab[j, j // C] = 1.0          # S_a: P0 row j -> h = j//8  (h in 0..15)
        sab[j, 32 + 16 + j // C] = 1.0  # S_b: P1 row j -> h = 16 + j//8
    d = {
        "wf1": np.tile(fW1, (4, 1)),
        "wf2": fW2,
        "wf3": fW3,
        "w0": w0,
        "sab": sab,
        "wi1": np.tile(np.concatenate([iW1, np.zeros((24, 64), np.float32)], 0), (4, 1)),
        "wi2": np.tile(iW2, (2, 1)),
        "wr1": np.tile(rW1, (4, 1)),
        "wr2": np.tile(np.concatenate([rW2.reshape(32, 1), np.zeros((32, 31), np.float32)], axis=1), (4, 1)),
        "fb1": fb1.reshape(128, 1),
        "fb2": fb2.reshape(128, 1),
        "fb3": fb3.reshape(2, 128).T.copy(),  # wait: fb3 is (256,) = j index; col t half
        "ib1": ib1.reshape(64, 1),
        "ib2": np.tile(ib2.reshape(32, 1), (4, 1)),
        "rb1": np.tile(rb1.reshape(32, 1), (4, 1)),
    }
    # fix fb3: column t should be fb3[t*128:(t+1)*128]
    fb3v = np.asarray(fb3, np.float32).reshape(256)
    d["fb3"] = np.stack([fb3v[0:128], fb3v[128:256]], axis=1).copy()
    return {k: np.ascontiguousarray(v, dtype=np.float32) for k, v in d.items()}


def _prep_coeffs(coeffs, n_pieces):
    """coeffs (B, NP, 32) -> per-core csA/csB (n_pieces, 64, 512) and a0 (2,64,512)."""
    npc = coeffs.shape[1]
    x = np.asarray(coeffs, np.float32).reshape(CORES, 2, 2, SUB, npc, 32)
    # slab[core, g, p, 32q+j, n] = x[core, g, q, n, p, 8+j]
    sl = x[..., 8:32]                                  # (8, 2, 2, 512, np, 24)
    sl = np.transpose(sl, (0, 1, 4, 2, 5, 3))          # (8, 2, np, 2, 24, 512)
    sl = np.pad(sl, ((0, 0),) * 4 + ((0, 8), (0, 0)))  # (8, 2, np, 2, 32, 512)
    sl = sl.reshape(CORES, 2, npc, 64, SUB)[:, :, :n_pieces]
    sl = np.ascontiguousarray(sl)
    # a0[core, g, 32q+cc, n] = coeffs[core,g,q,n, piece0, cc]
    a = x[:, :, :, :, 0, 0:8]                          # (8, 2, 2, 512, 8)
    a = np.transpose(a, (0, 1, 2, 4, 3))               # (8, 2, 2, 8, 512)
    a = np.pad(a, ((0, 0),) * 3 + ((0, 24), (0, 0)))   # (8, 2, 2, 32, 512)
    a = np.ascontiguousarray(a.reshape(CORES, 2, 64, SUB))
    return sl, a


def _unscramble_out(res_list, rb2, n_knots=L):
    """res_list: per-core dicts with 'out' (2,128,1024) -> (B, n_knots, 1)."""
    outs = []
    for c in range(CORES):
        o = res_list[c]["out"]                              # (2, 128, 2, 512): g, l, q, n
        o = np.transpose(o, (0, 2, 3, 1))                   # g, q, n, l
        outs.append(o.reshape(BC, 128))
    full = np.concatenate(outs, axis=0)[:, :n_knots]       # (B, L)
    return (full + np.float32(rb2.reshape(-1)[0])).astype(np.float32)[:, :, None]


LAST_RES = None


def kernel(coeffs, t_eval, iW1, ib1, iW2, ib2, fW1, fb1, fW2, fb2, fW3, fb3,
           rW1, rb1, rW2, rb2, _num_steps=STEPS, _n_pieces=NPIECE, _n_knots=L,
           _time_iters=0, _trace=False, _tmpdir=None):
    global LAST_RES
    nc = _build(_num_steps, _n_pieces, _n_knots)
    w = _prep_weights(iW1, ib1, iW2, ib2, fW1, fb1, fW2, fb2, fW3, fb3, rW1, rb1, rW2)
    sl, a0 = _prep_coeffs(coeffs, _n_pieces)
    in_maps = []
    for c in range(CORES):
        m = dict(w)
        m["csA"] = sl[c, 0]
        m["csB"] = sl[c, 1]
        m["a0"] = a0[c]
        in_maps.append(m)
    res = bass_utils.run_bass_kernel_spmd(
        nc, in_maps, core_ids=list(range(CORES)),
        trace=_trace, tmpdir=_tmpdir)
    LAST_RES = res
    return _unscramble_out(res.results, np.asarray(rb2), _n_knots)

